# revision 7
# baseline (speedup 1.0000x reference)
"""Multi-head graph attention (GAT-style) Trainium2 kernel.

Reference computation (fp32):
    h_prime = einsum('nf,hfo->hno', h, w)            # [H, N, O]
    attn_src = einsum('hno,ho->hn', h_prime, a_src)  # [H, N]
    attn_dst = einsum('hno,ho->hn', h_prime, a_dst)  # [H, N]
    score[h,i,j] = leaky_relu(attn_src[h,i] + attn_dst[h,j], 0.2)
    attn = softmax(score, axis=-1)                   # [H, N, N]
    output = attn @ h_prime + bias                   # [H, N, O]
    returns (output, attn)

Sharding: row-parallel over query rows. Each of the 8 cores owns N/8 = 768
query rows for all 4 heads, computes its rows of `attn` (the 604MB output
that dominates: memory-bound) and `output`, host concatenates.

Key identities used on device:
  leaky(x) = max(x, 0.2x)  and exp is monotone, so
  e = exp(leaky(src_i + dst_j)) = max(exp(src_i + dst_j), exp(0.2(src_i + dst_j)))
  -> two ScalarE Exp passes (per-partition bias = src_i / 0.2 src_i) + one
     VectorE max. Softmax max-subtraction is skipped (scores are O(+-10),
     well within fp32 exp range; matches reference to ~1e-6).
  Row sums S_i come for free from the `output` matmul by augmenting h_prime
  with a ones column (psum column 32 = sum_j e_ij).
"""
import os
import numpy as np
from contextlib import ExitStack

import concourse.bass as bass
import concourse.tile as tile
from concourse import bacc, mybir, bass_utils

N_HEAD, N, F_IN, F_OUT = 4, 6144, 64, 32
N_CORES = 8
ROWS = N // N_CORES          # 768 query rows per core
P = 128                      # partition dim
NBLK = ROWS // P             # 6 i-blocks per head per core
NCH = N // P                 # 48 key chunks of 128
FD = N                       # free dim of a full row tile
F32 = mybir.dt.float32

_CACHED_NC = None


def _build():
    nc = bacc.Bacc("TRN2", target_bir_lowering=False, debug=False,
                   num_devices=N_CORES)
    # ---- DRAM I/O ----
    h_d = nc.dram_tensor("h", (N, F_IN), F32, kind="ExternalInput").ap()
    hq_d = nc.dram_tensor("hq", (ROWS, F_IN), F32, kind="ExternalInput").ap()
    w_d = nc.dram_tensor("w", (N_HEAD, F_IN, F_OUT), F32, kind="ExternalInput").ap()
    asrc_d = nc.dram_tensor("a_src", (N_HEAD, F_OUT), F32, kind="ExternalInput").ap()
    adst_d = nc.dram_tensor("a_dst", (N_HEAD, F_OUT), F32, kind="ExternalInput").ap()
    bias_d = nc.dram_tensor("bias", (F_OUT,), F32, kind="ExternalInput").ap()
    ident_d = nc.dram_tensor("ident", (P, P), F32, kind="ExternalInput").ap()

    attn_d = nc.dram_tensor("attn_part", (N_HEAD, ROWS, N), F32,
                            kind="ExternalOutput").ap()
    out_d = nc.dram_tensor("out_part", (N_HEAD, ROWS, F_OUT), F32,
                           kind="ExternalOutput").ap()

    with tile.TileContext(nc) as tc, ExitStack() as ctx:
        # ---------------- persistent pools ----------------
        pers = ctx.enter_context(tc.tile_pool(name="pers", bufs=1))
        ident = pers.tile([P, P], F32)
        nc.sync.dma_start(ident[:], ident_d)

        # hp augmented with a ones column: per head [128, 48*33]
        hp_aug = [pers.tile([P, NCH * (F_OUT + 1)], F32, name=f"hp_aug{h}")
                  for h in range(N_HEAD)]
        # per head per i-block biases: src and 0.2*src, [128, NBLK]
        src_b = [pers.tile([P, NBLK], F32, name=f"src_b{h}") for h in range(N_HEAD)]
        src_b2 = [pers.tile([P, NBLK], F32, name=f"src_b2{h}") for h in range(N_HEAD)]
        # dst row for the current head (rebuilt per head) + persistent hT
        dst_row = pers.tile([1, N], F32)
        hT = pers.tile([F_IN, N], F32)
        u_cols = [pers.tile([F_IN, 1], F32, name=f"u_col{h}") for h in range(N_HEAD)]
        # bias broadcast [128, 32]
        bias_bc = pers.tile([P, F_OUT], F32)

        # ---------------- prep ----------------
        with tc.tile_pool(name="prep", bufs=1) as prep, \
             tc.tile_pool(name="prep_ps", bufs=1, space="PSUM") as prep_ps:
            # hT [64, N]: transpose h chunks
            for c in range(NCH):
                htile = prep.tile([P, F_IN], F32, tag="h_load")
                nc.sync.dma_start(htile[:], h_d[c * P:(c + 1) * P, :])
                ps = prep_ps.tile([F_IN, P], F32, tag="hT_ps")
                nc.tensor.transpose(ps[:], htile[:], ident[:])
                nc.scalar.copy(hT[:, c * P:(c + 1) * P], ps[:])
            # hqT [64, ROWS]
            hqT = prep.tile([F_IN, ROWS], F32)
            for c in range(NBLK):
                htile = prep.tile([P, F_IN], F32, tag="h_load")
                nc.sync.dma_start(htile[:], hq_d[c * P:(c + 1) * P, :])
                ps = prep_ps.tile([F_IN, P], F32, tag="hT_ps")
                nc.tensor.transpose(ps[:], htile[:], ident[:])
                nc.scalar.copy(hqT[:, c * P:(c + 1) * P], ps[:])
            # ones column of hp_aug
            for h in range(N_HEAD):
                nc.vector.memset(
                    hp_aug[h][:].rearrange("p (c k) -> p c k", k=F_OUT + 1)[:, :, F_OUT:F_OUT + 1],
                    1.0)
            # bias broadcast via ones-outer: [1,32] -> [128,32]
            bias_row = prep.tile([1, F_OUT], F32)
            nc.sync.dma_start(bias_row[:], bias_d.unsqueeze(0))
            ones1 = prep.tile([1, P], F32)
            nc.vector.memset(ones1[:], 1.0)
            ps_b = prep_ps.tile([P, F_OUT], F32, tag="bias_ps")
            nc.tensor.matmul(ps_b[:], ones1[:], bias_row[:], start=True, stop=True)
            nc.scalar.copy(bias_bc[:], ps_b[:])

            for h in range(N_HEAD):
                # w_h [64, 32]
                wt = prep.tile([F_IN, F_OUT], F32, tag="w_load")
                nc.sync.dma_start(wt[:], w_d[h])
                # hp chunks: lhsT = hT[:, chunk] [64,128], rhs = w_h -> [128, 32]
                for c in range(NCH):
                    ps = prep_ps.tile([P, F_OUT], F32, tag="hp_ps")
                    nc.tensor.matmul(ps[:], hT[:, c * P:(c + 1) * P], wt[:],
                                     start=True, stop=True)
                    nc.scalar.copy(
                        hp_aug[h][:, c * (F_OUT + 1):c * (F_OUT + 1) + F_OUT], ps[:])
                # v = w_h @ a_src, u = w_h @ a_dst  (via wT [32, 64])
                ps_wT = prep_ps.tile([F_OUT, F_IN], F32, tag="wT_ps")
                nc.tensor.transpose(ps_wT[:], wt[:], ident[0:F_IN, 0:F_IN])
                wT = prep.tile([F_OUT, F_IN], F32, tag="wT")
                nc.scalar.copy(wT[:], ps_wT[:])
                acol = prep.tile([F_OUT, 1], F32, tag="acol")
                nc.sync.dma_start(acol[:], asrc_d[h].unsqueeze(1))
                ps_v = prep_ps.tile([F_IN, 1], F32, tag="small_ps")
                nc.tensor.matmul(ps_v[:], wT[:], acol[:], start=True, stop=True)
                v = prep.tile([F_IN, 1], F32, tag="v")
                nc.scalar.copy(v[:], ps_v[:])
                acol2 = prep.tile([F_OUT, 1], F32, tag="acol2")
                nc.sync.dma_start(acol2[:], adst_d[h].unsqueeze(1))
                ps_u = prep_ps.tile([F_IN, 1], F32, tag="small_ps")
                nc.tensor.matmul(ps_u[:], wT[:], acol2[:], start=True, stop=True)
                nc.scalar.copy(u_cols[h][:], ps_u[:])
                # src per i-block: lhsT = hqT chunk [64, 128], rhs = v -> [128, 1]
                for b in range(NBLK):
                    ps = prep_ps.tile([P, 1], F32, tag="small_ps")
                    nc.tensor.matmul(ps[:], hqT[:, b * P:(b + 1) * P], v[:],
                                     start=True, stop=True)
                    nc.scalar.copy(src_b[h][:, b:b + 1], ps[:])
                nc.vector.tensor_scalar_mul(src_b2[h][:], src_b[h][:], 0.2)

        # ---------------- main loop pools ----------------
        mp = ctx.enter_context(tc.tile_pool(name="mp", bufs=1))
        e_pool = ctx.enter_context(tc.tile_pool(name="e", bufs=2))
        pq_pool = ctx.enter_context(tc.tile_pool(name="pq", bufs=2))
        eT_pool = ctx.enter_context(tc.tile_pool(name="eT", bufs=3))
        small = ctx.enter_context(tc.tile_pool(name="small", bufs=4))
        ps_tp = ctx.enter_context(tc.tile_pool(name="ps_tp", bufs=2, space="PSUM"))
        ps_out = ctx.enter_context(tc.tile_pool(name="ps_out", bufs=2, space="PSUM"))
        ps_bc = ctx.enter_context(tc.tile_pool(name="ps_bc", bufs=2, space="PSUM"))

        dst_bc = mp.tile([P, N], F32)          # per-head broadcast of dst row
        ones1m = mp.tile([1, P], F32)
        nc.vector.memset(ones1m[:], 1.0)

        JC = 1536  # j-chunk size for p/q temporaries

        for h in range(N_HEAD):
            # dst row for this head: lhsT = u [64, 1], rhs = hT [64, 512]
            for c in range(N // 512):
                ps = ps_bc.tile([1, 512], F32, tag="dstps")
                nc.tensor.matmul(ps[:], u_cols[h][:], hT[:, c * 512:(c + 1) * 512],
                                 start=True, stop=True)
                nc.scalar.copy(dst_row[:, c * 512:(c + 1) * 512], ps[:])
            # broadcast dst row -> [128, N] via ones-outer in 512-col chunks
            for c in range(N // 512):
                ps = ps_bc.tile([P, 512], F32, tag="bc")
                nc.tensor.matmul(ps[:], ones1m[:],
                                 dst_row[:, c * 512:(c + 1) * 512],
                                 start=True, stop=True)
                eng = nc.scalar if c % 2 == 0 else nc.vector
                if c % 2 == 0:
                    nc.scalar.copy(dst_bc[:, c * 512:(c + 1) * 512], ps[:])
                else:
                    nc.vector.tensor_copy(dst_bc[:, c * 512:(c + 1) * 512], ps[:])

            for b in range(NBLK):
                e_row = e_pool.tile([P, N], F32, tag="e_row")
                # generate e = max(exp(x), exp(0.2 x)), x = src_i + dst_j
                for jc in range(N // JC):
                    sl = slice(jc * JC, (jc + 1) * JC)
                    p_t = pq_pool.tile([P, JC], F32, tag="p_t")
                    nc.scalar.activation(p_t[:], dst_bc[:, sl],
                                         mybir.ActivationFunctionType.Exp,
                                         bias=src_b[h][:, b:b + 1], scale=1.0)
                    q_t = pq_pool.tile([P, JC], F32, tag="q_t")
                    nc.scalar.activation(q_t[:], dst_bc[:, sl],
                                         mybir.ActivationFunctionType.Exp,
                                         bias=src_b2[h][:, b:b + 1], scale=0.2)
                    nc.vector.tensor_tensor(e_row[:, sl], p_t[:], q_t[:],
                                            mybir.AluOpType.max)
                # transpose e chunks, evict, accumulate out-matmul
                ps_o = ps_out.tile([P, F_OUT + 1], F32, tag="po")
                for g in range(NCH // 4):     # groups of 4 transposes -> [128, 512]
                    ps_t = ps_tp.tile([P, 512], F32, tag="pt")
                    for k in range(4):
                        c = g * 4 + k
                        nc.tensor.transpose(ps_t[:, k * P:(k + 1) * P],
                                            e_row[:, c * P:(c + 1) * P], ident[:])
                    eT = eT_pool.tile([P, 512], F32, tag="eT")
                    if g % 2 == 0:
                        nc.scalar.copy(eT[:], ps_t[:])
                    else:
                        nc.vector.tensor_copy(eT[:], ps_t[:])
                    for k in range(4):
                        c = g * 4 + k
                        nc.tensor.matmul(
                            ps_o[:], eT[:, k * P:(k + 1) * P],
                            hp_aug[h][:, c * (F_OUT + 1):(c + 1) * (F_OUT + 1)],
                            start=(c == 0), stop=(c == NCH - 1))
                # S, r, normalize, outputs
                r = small.tile([P, 1], F32, tag="r")
                nc.vector.reciprocal(r[:], ps_o[:, F_OUT:F_OUT + 1])
                nc.vector.tensor_scalar(e_row[:], e_row[:], r[:], None,
                                        mybir.AluOpType.mult)
                nc.sync.dma_start(attn_d[h, b * P:(b + 1) * P, :], e_row[:])
                out_sc = small.tile([P, F_OUT], F32, tag="out_sc")
                nc.vector.tensor_scalar(out_sc[:], ps_o[:, 0:F_OUT], r[:], None,
                                        mybir.AluOpType.mult)
                out_fin = small.tile([P, F_OUT], F32, tag="out_fin")
                nc.vector.tensor_tensor(out_fin[:], out_sc[:], bias_bc[:],
                                        mybir.AluOpType.add)
                nc.sync.dma_start(out_d[h, b * P:(b + 1) * P, :], out_fin[:])

    nc.compile()
    return nc


def kernel(h, w, a_src, a_dst, bias):
    global _CACHED_NC
    h = np.ascontiguousarray(h, dtype=np.float32)
    w = np.ascontiguousarray(w, dtype=np.float32)
    a_src = np.ascontiguousarray(a_src, dtype=np.float32)
    a_dst = np.ascontiguousarray(a_dst, dtype=np.float32)
    bias = np.ascontiguousarray(bias, dtype=np.float32)

    if _CACHED_NC is None:
        _CACHED_NC = _build()
    nc = _CACHED_NC

    ident = np.eye(P, dtype=np.float32)
    in_maps = []
    for c in range(N_CORES):
        in_maps.append({
            "h": h,
            "hq": h[c * ROWS:(c + 1) * ROWS],
            "w": w,
            "a_src": a_src,
            "a_dst": a_dst,
            "bias": bias,
            "ident": ident,
        })
    res = bass_utils.run_bass_kernel_spmd(nc, in_maps, core_ids=list(range(N_CORES)))
    attn = np.concatenate([r["attn_part"] for r in res.results], axis=1)
    output = np.concatenate([r["out_part"] for r in res.results], axis=1)
    return (output, attn)


if __name__ == "__main__":
    # quick self-run with random inputs
    rng = np.random.default_rng(0)
    h = rng.standard_normal((N, F_IN)).astype(np.float32)
    w = (rng.standard_normal((N_HEAD, F_IN, F_OUT)) * 0.15).astype(np.float32)
    a_src = (rng.standard_normal((N_HEAD, F_OUT)) * 0.2).astype(np.float32)
    a_dst = (rng.standard_normal((N_HEAD, F_OUT)) * 0.2).astype(np.float32)
    bias = np.zeros(F_OUT, dtype=np.float32)
    out, attn = kernel(h=h, w=w, a_src=a_src, a_dst=a_dst, bias=bias)
    print("out", out.shape, "attn", attn.shape, attn[0, 0, :4])


# revision 11
# speedup vs baseline: 1.0446x; 1.0446x over previous
"""Multi-head graph attention (GAT-style) Trainium2 kernel.

Reference computation (fp32):
    h_prime = einsum('nf,hfo->hno', h, w)            # [H, N, O]
    attn_src = einsum('hno,ho->hn', h_prime, a_src)  # [H, N]
    attn_dst = einsum('hno,ho->hn', h_prime, a_dst)  # [H, N]
    score[h,i,j] = leaky_relu(attn_src[h,i] + attn_dst[h,j], 0.2)
    attn = softmax(score, axis=-1)                   # [H, N, N]
    output = attn @ h_prime + bias                   # [H, N, O]
    returns (output, attn)

Sharding: row-parallel over query rows. Each of the 8 cores owns N/8 = 768
query rows for all 4 heads, computes its rows of `attn` (the 604MB output
that dominates: memory-bound) and `output`, host concatenates.

Key identities used on device:
  leaky(x) = max(x, 0.2x)  and exp is monotone, so
  e = exp(leaky(src_i + dst_j)) = max(exp(src_i + dst_j), exp(0.2(src_i + dst_j)))
  -> two ScalarE Exp passes (per-partition bias = src_i / 0.2 src_i) + one
     VectorE max. Softmax max-subtraction is skipped (scores are O(+-10),
     well within fp32 exp range; matches reference to ~1e-6).
  Row sums S_i come for free from the `output` matmul by augmenting h_prime
  with a ones column (psum column 32 = sum_j e_ij).
"""
import os
import numpy as np
from contextlib import ExitStack

import concourse.bass as bass
import concourse.tile as tile
from concourse import bacc, mybir, bass_utils

N_HEAD, N, F_IN, F_OUT = 4, 6144, 64, 32
N_CORES = 8
ROWS = N // N_CORES          # 768 query rows per core
P = 128                      # partition dim
NBLK = ROWS // P             # 6 i-blocks per head per core
NCH = N // P                 # 48 key chunks of 128
FD = N                       # free dim of a full row tile
F32 = mybir.dt.float32

_CACHED_NC = None


def _build():
    nc = bacc.Bacc("TRN2", target_bir_lowering=False, debug=False,
                   num_devices=N_CORES)
    # ---- DRAM I/O ----
    h_d = nc.dram_tensor("h", (N, F_IN), F32, kind="ExternalInput").ap()
    hq_d = nc.dram_tensor("hq", (ROWS, F_IN), F32, kind="ExternalInput").ap()
    w_d = nc.dram_tensor("w", (N_HEAD, F_IN, F_OUT), F32, kind="ExternalInput").ap()
    asrc_d = nc.dram_tensor("a_src", (N_HEAD, F_OUT), F32, kind="ExternalInput").ap()
    adst_d = nc.dram_tensor("a_dst", (N_HEAD, F_OUT), F32, kind="ExternalInput").ap()
    bias_d = nc.dram_tensor("bias", (F_OUT,), F32, kind="ExternalInput").ap()
    ident_d = nc.dram_tensor("ident", (P, P), F32, kind="ExternalInput").ap()

    attn_d = nc.dram_tensor("attn_part", (N_HEAD, ROWS, N), F32,
                            kind="ExternalOutput").ap()
    out_d = nc.dram_tensor("out_part", (N_HEAD, ROWS, F_OUT), F32,
                           kind="ExternalOutput").ap()

    with tile.TileContext(nc) as tc, ExitStack() as ctx:
        # ---------------- persistent pools ----------------
        pers = ctx.enter_context(tc.tile_pool(name="pers", bufs=1))
        ident = pers.tile([P, P], F32)
        nc.sync.dma_start(ident[:], ident_d)

        # hp augmented with a ones column: per head [128, 48*33]
        hp_aug = [pers.tile([P, NCH * (F_OUT + 1)], F32, name=f"hp_aug{h}")
                  for h in range(N_HEAD)]
        # per head per i-block biases: src and 0.2*src, [128, NBLK]
        src_b = [pers.tile([P, NBLK], F32, name=f"src_b{h}") for h in range(N_HEAD)]
        src_b2 = [pers.tile([P, NBLK], F32, name=f"src_b2{h}") for h in range(N_HEAD)]
        # persistent hT (dst row is built in transient 512-chunks per head)
        hT = pers.tile([F_IN, N], F32)
        u_cols = [pers.tile([F_IN, 1], F32, name=f"u_col{h}") for h in range(N_HEAD)]
        # bias broadcast [128, 32]
        bias_bc = pers.tile([P, F_OUT], F32)

        # ---------------- prep ----------------
        with tc.tile_pool(name="prep", bufs=1) as prep, \
             tc.tile_pool(name="prep_ps", bufs=1, space="PSUM") as prep_ps:
            # hT [64, N]: transpose h chunks
            for c in range(NCH):
                htile = prep.tile([P, F_IN], F32, tag="h_load")
                nc.sync.dma_start(htile[:], h_d[c * P:(c + 1) * P, :])
                ps = prep_ps.tile([F_IN, P], F32, tag="hT_ps")
                nc.tensor.transpose(ps[:], htile[:], ident[:])
                nc.scalar.copy(hT[:, c * P:(c + 1) * P], ps[:])
            # hqT [64, ROWS]
            hqT = prep.tile([F_IN, ROWS], F32)
            for c in range(NBLK):
                htile = prep.tile([P, F_IN], F32, tag="h_load")
                nc.sync.dma_start(htile[:], hq_d[c * P:(c + 1) * P, :])
                ps = prep_ps.tile([F_IN, P], F32, tag="hT_ps")
                nc.tensor.transpose(ps[:], htile[:], ident[:])
                nc.scalar.copy(hqT[:, c * P:(c + 1) * P], ps[:])
            # ones column of hp_aug
            for h in range(N_HEAD):
                nc.vector.memset(
                    hp_aug[h][:].rearrange("p (c k) -> p c k", k=F_OUT + 1)[:, :, F_OUT:F_OUT + 1],
                    1.0)
            # bias broadcast via ones-outer: [1,32] -> [128,32]
            bias_row = prep.tile([1, F_OUT], F32)
            nc.sync.dma_start(bias_row[:], bias_d.unsqueeze(0))
            ones1 = prep.tile([1, P], F32)
            nc.vector.memset(ones1[:], 1.0)
            ps_b = prep_ps.tile([P, F_OUT], F32, tag="bias_ps")
            nc.tensor.matmul(ps_b[:], ones1[:], bias_row[:], start=True, stop=True)
            nc.scalar.copy(bias_bc[:], ps_b[:])

            for h in range(N_HEAD):
                # w_h [64, 32]
                wt = prep.tile([F_IN, F_OUT], F32, tag="w_load")
                nc.sync.dma_start(wt[:], w_d[h])
                # hp chunks: lhsT = hT[:, chunk] [64,128], rhs = w_h -> [128, 32]
                for c in range(NCH):
                    ps = prep_ps.tile([P, F_OUT], F32, tag="hp_ps")
                    nc.tensor.matmul(ps[:], hT[:, c * P:(c + 1) * P], wt[:],
                                     start=True, stop=True)
                    nc.scalar.copy(
                        hp_aug[h][:, c * (F_OUT + 1):c * (F_OUT + 1) + F_OUT], ps[:])
                # v = w_h @ a_src, u = w_h @ a_dst  (via wT [32, 64])
                ps_wT = prep_ps.tile([F_OUT, F_IN], F32, tag="wT_ps")
                nc.tensor.transpose(ps_wT[:], wt[:], ident[0:F_IN, 0:F_IN])
                wT = prep.tile([F_OUT, F_IN], F32, tag="wT")
                nc.scalar.copy(wT[:], ps_wT[:])
                acol = prep.tile([F_OUT, 1], F32, tag="acol")
                nc.sync.dma_start(acol[:], asrc_d[h].unsqueeze(1))
                ps_v = prep_ps.tile([F_IN, 1], F32, tag="small_ps")
                nc.tensor.matmul(ps_v[:], wT[:], acol[:], start=True, stop=True)
                v = prep.tile([F_IN, 1], F32, tag="v")
                nc.scalar.copy(v[:], ps_v[:])
                acol2 = prep.tile([F_OUT, 1], F32, tag="acol2")
                nc.sync.dma_start(acol2[:], adst_d[h].unsqueeze(1))
                ps_u = prep_ps.tile([F_IN, 1], F32, tag="small_ps")
                nc.tensor.matmul(ps_u[:], wT[:], acol2[:], start=True, stop=True)
                nc.scalar.copy(u_cols[h][:], ps_u[:])
                # src per i-block: lhsT = hqT chunk [64, 128], rhs = v -> [128, 1]
                for b in range(NBLK):
                    ps = prep_ps.tile([P, 1], F32, tag="small_ps")
                    nc.tensor.matmul(ps[:], hqT[:, b * P:(b + 1) * P], v[:],
                                     start=True, stop=True)
                    nc.scalar.copy(src_b[h][:, b:b + 1], ps[:])
                nc.vector.tensor_scalar_mul(src_b2[h][:], src_b[h][:], 0.2)

        # ---------------- main loop pools ----------------
        mp = ctx.enter_context(tc.tile_pool(name="mp", bufs=1))
        e_pool = ctx.enter_context(tc.tile_pool(name="e", bufs=3))
        pq_pool = ctx.enter_context(tc.tile_pool(name="pq", bufs=2))
        eT_pool = ctx.enter_context(tc.tile_pool(name="eT", bufs=2))
        small = ctx.enter_context(tc.tile_pool(name="small", bufs=4))
        ps_tp = ctx.enter_context(tc.tile_pool(name="ps_tp", bufs=2, space="PSUM"))
        ps_out = ctx.enter_context(tc.tile_pool(name="ps_out", bufs=2, space="PSUM"))
        ps_bc = ctx.enter_context(tc.tile_pool(name="ps_bc", bufs=1, space="PSUM"))

        dst_bc = mp.tile([P, N], F32)          # per-head broadcast of dst row
        ones1m = mp.tile([1, P], F32)
        nc.vector.memset(ones1m[:], 1.0)

        JC = 1536  # j-chunk size for p/q temporaries

        for h in range(N_HEAD):
            # dst row chunk ([1,512]) then broadcast to [128,512] via ones-outer
            for c in range(N // 512):
                ps_d = ps_bc.tile([1, 512], F32, tag="dstps")
                nc.tensor.matmul(ps_d[:], u_cols[h][:], hT[:, c * 512:(c + 1) * 512],
                                 start=True, stop=True)
                drow = small.tile([1, 512], F32, tag="drow", bufs=2)
                nc.scalar.copy(drow[:], ps_d[:])
                ps = ps_bc.tile([P, 512], F32, tag="bc")
                nc.tensor.matmul(ps[:], ones1m[:], drow[:],
                                 start=True, stop=True)
                eng = nc.scalar if c % 2 == 0 else nc.vector
                if c % 2 == 0:
                    nc.scalar.copy(dst_bc[:, c * 512:(c + 1) * 512], ps[:])
                else:
                    nc.vector.tensor_copy(dst_bc[:, c * 512:(c + 1) * 512], ps[:])

            for b in range(NBLK):
                e_row = e_pool.tile([P, N], F32, tag="e_row")
                # generate e = max(exp(x), exp(0.2 x)), x = src_i + dst_j
                for jc in range(N // JC):
                    sl = slice(jc * JC, (jc + 1) * JC)
                    p_t = pq_pool.tile([P, JC], F32, tag="p_t")
                    nc.scalar.activation(p_t[:], dst_bc[:, sl],
                                         mybir.ActivationFunctionType.Exp,
                                         bias=src_b[h][:, b:b + 1], scale=1.0)
                    q_t = pq_pool.tile([P, JC], F32, tag="q_t")
                    nc.scalar.activation(q_t[:], dst_bc[:, sl],
                                         mybir.ActivationFunctionType.Exp,
                                         bias=src_b2[h][:, b:b + 1], scale=0.2)
                    nc.vector.tensor_tensor(e_row[:, sl], p_t[:], q_t[:],
                                            mybir.AluOpType.max)
                # transpose e chunks, evict, accumulate out-matmul
                ps_o = ps_out.tile([P, F_OUT + 1], F32, tag="po")
                for g in range(NCH // 8):     # groups of 8 transposes -> [128, 1024]
                    ps_t = ps_tp.tile([P, 1024], F32, tag="pt")
                    for k in range(8):
                        c = g * 8 + k
                        nc.tensor.transpose(ps_t[:, k * P:(k + 1) * P],
                                            e_row[:, c * P:(c + 1) * P], ident[:])
                    eT = eT_pool.tile([P, 1024], F32, tag="eT")
                    if g % 2 == 0:
                        nc.scalar.copy(eT[:], ps_t[:])
                    else:
                        nc.vector.tensor_copy(eT[:], ps_t[:])
                    for k in range(8):
                        c = g * 8 + k
                        nc.tensor.matmul(
                            ps_o[:], eT[:, k * P:(k + 1) * P],
                            hp_aug[h][:, c * (F_OUT + 1):(c + 1) * (F_OUT + 1)],
                            start=(c == 0), stop=(c == NCH - 1))
                # S, r, normalize, outputs
                r = small.tile([P, 1], F32, tag="r")
                nc.vector.reciprocal(r[:], ps_o[:, F_OUT:F_OUT + 1])
                nc.vector.tensor_scalar(e_row[:], e_row[:], r[:], None,
                                        mybir.AluOpType.mult)
                nc.sync.dma_start(attn_d[h, b * P:(b + 1) * P, :], e_row[:])
                out_sc = small.tile([P, F_OUT], F32, tag="out_sc")
                nc.vector.tensor_scalar(out_sc[:], ps_o[:, 0:F_OUT], r[:], None,
                                        mybir.AluOpType.mult)
                out_fin = small.tile([P, F_OUT], F32, tag="out_fin")
                nc.vector.tensor_tensor(out_fin[:], out_sc[:], bias_bc[:],
                                        mybir.AluOpType.add)
                nc.sync.dma_start(out_d[h, b * P:(b + 1) * P, :], out_fin[:])

    nc.compile()
    return nc


def kernel(h, w, a_src, a_dst, bias):
    global _CACHED_NC
    h = np.ascontiguousarray(h, dtype=np.float32)
    w = np.ascontiguousarray(w, dtype=np.float32)
    a_src = np.ascontiguousarray(a_src, dtype=np.float32)
    a_dst = np.ascontiguousarray(a_dst, dtype=np.float32)
    bias = np.ascontiguousarray(bias, dtype=np.float32)

    if _CACHED_NC is None:
        _CACHED_NC = _build()
    nc = _CACHED_NC

    ident = np.eye(P, dtype=np.float32)
    in_maps = []
    for c in range(N_CORES):
        in_maps.append({
            "h": h,
            "hq": h[c * ROWS:(c + 1) * ROWS],
            "w": w,
            "a_src": a_src,
            "a_dst": a_dst,
            "bias": bias,
            "ident": ident,
        })
    res = bass_utils.run_bass_kernel_spmd(nc, in_maps, core_ids=list(range(N_CORES)))
    attn = np.concatenate([r["attn_part"] for r in res.results], axis=1)
    output = np.concatenate([r["out_part"] for r in res.results], axis=1)
    return (output, attn)


if __name__ == "__main__":
    # quick self-run with random inputs
    rng = np.random.default_rng(0)
    h = rng.standard_normal((N, F_IN)).astype(np.float32)
    w = (rng.standard_normal((N_HEAD, F_IN, F_OUT)) * 0.15).astype(np.float32)
    a_src = (rng.standard_normal((N_HEAD, F_OUT)) * 0.2).astype(np.float32)
    a_dst = (rng.standard_normal((N_HEAD, F_OUT)) * 0.2).astype(np.float32)
    bias = np.zeros(F_OUT, dtype=np.float32)
    out, attn = kernel(h=h, w=w, a_src=a_src, a_dst=a_dst, bias=bias)
    print("out", out.shape, "attn", attn.shape, attn[0, 0, :4])


# revision 16
# speedup vs baseline: 1.1150x; 1.0674x over previous
"""Multi-head graph attention (GAT-style) Trainium2 kernel.

Reference computation (fp32):
    h_prime = einsum('nf,hfo->hno', h, w)            # [H, N, O]
    attn_src = einsum('hno,ho->hn', h_prime, a_src)  # [H, N]
    attn_dst = einsum('hno,ho->hn', h_prime, a_dst)  # [H, N]
    score[h,i,j] = leaky_relu(attn_src[h,i] + attn_dst[h,j], 0.2)
    attn = softmax(score, axis=-1)                   # [H, N, N]
    output = attn @ h_prime + bias                   # [H, N, O]
    returns (output, attn)

Sharding: row-parallel over query rows. Each of the 8 cores owns N/8 = 768
query rows for all 4 heads, computes its rows of `attn` (the 604MB output
that dominates: memory-bound) and `output`, host concatenates.

Key identities used on device:
  leaky(x) = max(x, 0.2x)  and exp is monotone, so
  e = exp(leaky(src_i + dst_j)) = max(exp(src_i + dst_j), exp(0.2(src_i + dst_j)))
  -> two ScalarE Exp passes (per-partition bias = src_i / 0.2 src_i) + one
     VectorE max. Softmax max-subtraction is skipped (scores are O(+-10),
     well within fp32 exp range; matches reference to ~1e-6).
  Row sums S_i come for free from the `output` matmul by augmenting h_prime
  with a ones column (psum column 32 = sum_j e_ij).
"""
import os
import numpy as np
from contextlib import ExitStack

import concourse.bass as bass
import concourse.tile as tile
from concourse import bacc, mybir, bass_utils

N_HEAD, N, F_IN, F_OUT = 4, 6144, 64, 32
N_CORES = 8
ROWS = N // N_CORES          # 768 query rows per core
P = 128                      # partition dim
NBLK = ROWS // P             # 6 i-blocks per head per core
NCH = N // P                 # 48 key chunks of 128
FD = N                       # free dim of a full row tile
F32 = mybir.dt.float32

_CACHED_NC = None


def _build():
    nc = bacc.Bacc("TRN2", target_bir_lowering=False, debug=False,
                   num_devices=N_CORES)
    # ---- DRAM I/O ----
    h_d = nc.dram_tensor("h", (N, F_IN), F32, kind="ExternalInput").ap()
    hq_d = nc.dram_tensor("hq", (ROWS, F_IN), F32, kind="ExternalInput").ap()
    w_d = nc.dram_tensor("w", (N_HEAD, F_IN, F_OUT), F32, kind="ExternalInput").ap()
    asrc_d = nc.dram_tensor("a_src", (N_HEAD, F_OUT), F32, kind="ExternalInput").ap()
    adst_d = nc.dram_tensor("a_dst", (N_HEAD, F_OUT), F32, kind="ExternalInput").ap()
    bias_d = nc.dram_tensor("bias", (F_OUT,), F32, kind="ExternalInput").ap()
    ident_d = nc.dram_tensor("ident", (P, P), F32, kind="ExternalInput").ap()

    attn_d = nc.dram_tensor("attn_part", (N_HEAD, ROWS, N), F32,
                            kind="ExternalOutput").ap()
    out_d = nc.dram_tensor("out_part", (N_HEAD, ROWS, F_OUT), F32,
                           kind="ExternalOutput").ap()

    with tile.TileContext(nc) as tc, ExitStack() as ctx:
        # ---------------- persistent pools ----------------
        pers = ctx.enter_context(tc.tile_pool(name="pers", bufs=1))
        ident = pers.tile([P, P], F32)
        nc.sync.dma_start(ident[:], ident_d)

        # hp augmented with a ones column: per head [128, 48*33]
        hp_aug = [pers.tile([P, NCH * (F_OUT + 1)], F32, name=f"hp_aug{h}")
                  for h in range(N_HEAD)]
        # per head per i-block biases: src and 0.2*src, [128, NBLK]
        src_b = [pers.tile([P, NBLK], F32, name=f"src_b{h}") for h in range(N_HEAD)]
        src_b2 = [pers.tile([P, NBLK], F32, name=f"src_b2{h}") for h in range(N_HEAD)]
        # persistent hT (dst row is built in transient 512-chunks per head)
        hT = pers.tile([F_IN, N], F32)
        u_cols = [pers.tile([F_IN, 1], F32, name=f"u_col{h}") for h in range(N_HEAD)]
        # bias broadcast [128, 32]
        bias_bc = pers.tile([P, F_OUT], F32)

        # ---------------- prep ----------------
        with tc.tile_pool(name="prep", bufs=1) as prep, \
             tc.tile_pool(name="prep_ps", bufs=2, space="PSUM") as prep_ps:
            # hT [64, N]: transpose h chunks
            for c in range(NCH):
                htile = prep.tile([P, F_IN], F32, tag="h_load")
                nc.sync.dma_start(htile[:], h_d[c * P:(c + 1) * P, :])
                ps = prep_ps.tile([F_IN, P], F32, tag="hT_ps")
                nc.tensor.transpose(ps[:], htile[:], ident[:])
                if c % 2 == 0:
                    nc.scalar.copy(hT[:, c * P:(c + 1) * P], ps[:])
                else:
                    nc.vector.tensor_copy(hT[:, c * P:(c + 1) * P], ps[:])
            # hqT [64, ROWS]
            hqT = prep.tile([F_IN, ROWS], F32)
            for c in range(NBLK):
                htile = prep.tile([P, F_IN], F32, tag="h_load")
                nc.sync.dma_start(htile[:], hq_d[c * P:(c + 1) * P, :])
                ps = prep_ps.tile([F_IN, P], F32, tag="hT_ps")
                nc.tensor.transpose(ps[:], htile[:], ident[:])
                nc.scalar.copy(hqT[:, c * P:(c + 1) * P], ps[:])
            # ones column of hp_aug
            for h in range(N_HEAD):
                nc.vector.memset(
                    hp_aug[h][:].rearrange("p (c k) -> p c k", k=F_OUT + 1)[:, :, F_OUT:F_OUT + 1],
                    1.0)
            # bias broadcast via ones-outer: [1,32] -> [128,32]
            bias_row = prep.tile([1, F_OUT], F32)
            nc.sync.dma_start(bias_row[:], bias_d.unsqueeze(0))
            ones1 = prep.tile([1, P], F32)
            nc.vector.memset(ones1[:], 1.0)
            ps_b = prep_ps.tile([P, F_OUT], F32, tag="small_ps")
            nc.tensor.matmul(ps_b[:], ones1[:], bias_row[:], start=True, stop=True)
            nc.scalar.copy(bias_bc[:], ps_b[:])

            for h in range(N_HEAD):
                # w_h [64, 32]
                wt = prep.tile([F_IN, F_OUT], F32, tag="w_load")
                nc.sync.dma_start(wt[:], w_d[h])
                # hp chunks: lhsT = hT[:, chunk] [64,128], rhs = w_h -> [128, 32]
                for c in range(NCH):
                    ps = prep_ps.tile([P, F_OUT], F32, tag="hp_ps")
                    nc.tensor.matmul(ps[:], hT[:, c * P:(c + 1) * P], wt[:],
                                     start=True, stop=True)
                    if c % 2 == 0:
                        nc.scalar.copy(
                            hp_aug[h][:, c * (F_OUT + 1):c * (F_OUT + 1) + F_OUT],
                            ps[:])
                    else:
                        nc.vector.tensor_copy(
                            hp_aug[h][:, c * (F_OUT + 1):c * (F_OUT + 1) + F_OUT],
                            ps[:])
                # v = w_h @ a_src, u = w_h @ a_dst  (via wT [32, 64])
                ps_wT = prep_ps.tile([F_OUT, F_IN], F32, tag="small_ps")
                nc.tensor.transpose(ps_wT[:], wt[:], ident[0:F_IN, 0:F_IN])
                wT = prep.tile([F_OUT, F_IN], F32, tag="wT")
                nc.scalar.copy(wT[:], ps_wT[:])
                acol = prep.tile([F_OUT, 1], F32, tag="acol")
                nc.sync.dma_start(acol[:], asrc_d[h].unsqueeze(1))
                ps_v = prep_ps.tile([F_IN, 1], F32, tag="small_ps")
                nc.tensor.matmul(ps_v[:], wT[:], acol[:], start=True, stop=True)
                v = prep.tile([F_IN, 1], F32, tag="v")
                nc.scalar.copy(v[:], ps_v[:])
                acol2 = prep.tile([F_OUT, 1], F32, tag="acol2")
                nc.sync.dma_start(acol2[:], adst_d[h].unsqueeze(1))
                ps_u = prep_ps.tile([F_IN, 1], F32, tag="small_ps")
                nc.tensor.matmul(ps_u[:], wT[:], acol2[:], start=True, stop=True)
                nc.scalar.copy(u_cols[h][:], ps_u[:])
                # src per i-block: lhsT = hqT chunk [64, 128], rhs = v -> [128, 1]
                for b in range(NBLK):
                    ps = prep_ps.tile([P, 1], F32, tag="small_ps")
                    nc.tensor.matmul(ps[:], hqT[:, b * P:(b + 1) * P], v[:],
                                     start=True, stop=True)
                    nc.scalar.copy(src_b[h][:, b:b + 1], ps[:])
                nc.vector.tensor_scalar_mul(src_b2[h][:], src_b[h][:], 0.2)

        # ---------------- main loop pools ----------------
        mp = ctx.enter_context(tc.tile_pool(name="mp", bufs=1))
        e_pool = ctx.enter_context(tc.tile_pool(name="e", bufs=3))
        pq_pool = ctx.enter_context(tc.tile_pool(name="pq", bufs=2))
        eT_pool = ctx.enter_context(tc.tile_pool(name="eT", bufs=2))
        small = ctx.enter_context(tc.tile_pool(name="small", bufs=4))
        ps_tp = ctx.enter_context(tc.tile_pool(name="ps_tp", bufs=2, space="PSUM"))
        ps_out = ctx.enter_context(tc.tile_pool(name="ps_out", bufs=2, space="PSUM"))
        ps_bc = ctx.enter_context(tc.tile_pool(name="ps_bc", bufs=1, space="PSUM"))

        dst_bc = mp.tile([P, N], F32)          # per-head broadcast of dst row
        ones1m = mp.tile([1, P], F32)
        nc.vector.memset(ones1m[:], 1.0)

        JC = 1536  # j-chunk size for p/q temporaries

        for h in range(N_HEAD):
            # dst row chunk ([1,512]) then broadcast to [128,512] via ones-outer
            for c in range(N // 512):
                ps_d = ps_bc.tile([1, 512], F32, tag="dstps")
                nc.tensor.matmul(ps_d[:], u_cols[h][:], hT[:, c * 512:(c + 1) * 512],
                                 start=True, stop=True)
                drow = small.tile([1, 512], F32, tag="drow", bufs=2)
                nc.scalar.copy(drow[:], ps_d[:])
                ps = ps_bc.tile([P, 512], F32, tag="bc")
                nc.tensor.matmul(ps[:], ones1m[:], drow[:],
                                 start=True, stop=True)
                eng = nc.scalar if c % 2 == 0 else nc.vector
                if c % 2 == 0:
                    nc.scalar.copy(dst_bc[:, c * 512:(c + 1) * 512], ps[:])
                else:
                    nc.vector.tensor_copy(dst_bc[:, c * 512:(c + 1) * 512], ps[:])

            for b in range(NBLK):
                e_row = e_pool.tile([P, N], F32, tag="e_row")
                # generate e = max(exp(x), exp(0.2 x)), x = src_i + dst_j
                for jc in range(N // JC):
                    sl = slice(jc * JC, (jc + 1) * JC)
                    p_t = pq_pool.tile([P, JC], F32, tag="p_t")
                    nc.scalar.activation(p_t[:], dst_bc[:, sl],
                                         mybir.ActivationFunctionType.Exp,
                                         bias=src_b[h][:, b:b + 1], scale=1.0)
                    q_t = pq_pool.tile([P, JC], F32, tag="q_t")
                    nc.scalar.activation(q_t[:], dst_bc[:, sl],
                                         mybir.ActivationFunctionType.Exp,
                                         bias=src_b2[h][:, b:b + 1], scale=0.2)
                    nc.vector.tensor_tensor(e_row[:, sl], p_t[:], q_t[:],
                                            mybir.AluOpType.max)
                # transpose e chunks, evict, accumulate out-matmul
                ps_o = ps_out.tile([P, F_OUT + 1], F32, tag="po")
                for g in range(NCH // 8):     # groups of 8 transposes -> [128, 1024]
                    ps_t = ps_tp.tile([P, 1024], F32, tag="pt")
                    for k in range(8):
                        c = g * 8 + k
                        nc.tensor.transpose(ps_t[:, k * P:(k + 1) * P],
                                            e_row[:, c * P:(c + 1) * P], ident[:])
                    eT = eT_pool.tile([P, 1024], F32, tag="eT")
                    if g % 2 == 0:
                        nc.scalar.copy(eT[:], ps_t[:])
                    else:
                        nc.vector.tensor_copy(eT[:], ps_t[:])
                    for k in range(8):
                        c = g * 8 + k
                        nc.tensor.matmul(
                            ps_o[:], eT[:, k * P:(k + 1) * P],
                            hp_aug[h][:, c * (F_OUT + 1):(c + 1) * (F_OUT + 1)],
                            start=(c == 0), stop=(c == NCH - 1))
                # S, r, normalize, outputs
                r = small.tile([P, 1], F32, tag="r")
                nc.vector.reciprocal(r[:], ps_o[:, F_OUT:F_OUT + 1])
                nc.vector.tensor_scalar(e_row[:], e_row[:], r[:], None,
                                        mybir.AluOpType.mult)
                nc.sync.dma_start(attn_d[h, b * P:(b + 1) * P, :], e_row[:])
                out_sc = small.tile([P, F_OUT], F32, tag="out_sc")
                nc.vector.tensor_scalar(out_sc[:], ps_o[:, 0:F_OUT], r[:], None,
                                        mybir.AluOpType.mult)
                out_fin = small.tile([P, F_OUT], F32, tag="out_fin")
                nc.vector.tensor_tensor(out_fin[:], out_sc[:], bias_bc[:],
                                        mybir.AluOpType.add)
                nc.sync.dma_start(out_d[h, b * P:(b + 1) * P, :], out_fin[:])

    nc.compile()
    return nc


def kernel(h, w, a_src, a_dst, bias):
    global _CACHED_NC
    h = np.ascontiguousarray(h, dtype=np.float32)
    w = np.ascontiguousarray(w, dtype=np.float32)
    a_src = np.ascontiguousarray(a_src, dtype=np.float32)
    a_dst = np.ascontiguousarray(a_dst, dtype=np.float32)
    bias = np.ascontiguousarray(bias, dtype=np.float32)

    if _CACHED_NC is None:
        _CACHED_NC = _build()
    nc = _CACHED_NC

    ident = np.eye(P, dtype=np.float32)
    in_maps = []
    for c in range(N_CORES):
        in_maps.append({
            "h": h,
            "hq": h[c * ROWS:(c + 1) * ROWS],
            "w": w,
            "a_src": a_src,
            "a_dst": a_dst,
            "bias": bias,
            "ident": ident,
        })
    res = bass_utils.run_bass_kernel_spmd(nc, in_maps, core_ids=list(range(N_CORES)))
    attn = np.concatenate([r["attn_part"] for r in res.results], axis=1)
    output = np.concatenate([r["out_part"] for r in res.results], axis=1)
    return (output, attn)


if __name__ == "__main__":
    # quick self-run with random inputs
    rng = np.random.default_rng(0)
    h = rng.standard_normal((N, F_IN)).astype(np.float32)
    w = (rng.standard_normal((N_HEAD, F_IN, F_OUT)) * 0.15).astype(np.float32)
    a_src = (rng.standard_normal((N_HEAD, F_OUT)) * 0.2).astype(np.float32)
    a_dst = (rng.standard_normal((N_HEAD, F_OUT)) * 0.2).astype(np.float32)
    bias = np.zeros(F_OUT, dtype=np.float32)
    out, attn = kernel(h=h, w=w, a_src=a_src, a_dst=a_dst, bias=bias)
    print("out", out.shape, "attn", attn.shape, attn[0, 0, :4])


# revision 18
# speedup vs baseline: 1.1472x; 1.0289x over previous
"""Multi-head graph attention (GAT-style) Trainium2 kernel.

Reference computation (fp32):
    h_prime = einsum('nf,hfo->hno', h, w)            # [H, N, O]
    attn_src = einsum('hno,ho->hn', h_prime, a_src)  # [H, N]
    attn_dst = einsum('hno,ho->hn', h_prime, a_dst)  # [H, N]
    score[h,i,j] = leaky_relu(attn_src[h,i] + attn_dst[h,j], 0.2)
    attn = softmax(score, axis=-1)                   # [H, N, N]
    output = attn @ h_prime + bias                   # [H, N, O]
    returns (output, attn)

Sharding: row-parallel over query rows. Each of the 8 cores owns N/8 = 768
query rows for all 4 heads, computes its rows of `attn` (the 604MB output
that dominates: memory-bound) and `output`, host concatenates.

Key identities used on device:
  leaky(x) = max(x, 0.2x)  and exp is monotone, so
  e = exp(leaky(src_i + dst_j)) = max(exp(src_i + dst_j), exp(0.2(src_i + dst_j)))
  -> two ScalarE Exp passes (per-partition bias = src_i / 0.2 src_i) + one
     VectorE max. Softmax max-subtraction is skipped (scores are O(+-10),
     well within fp32 exp range; matches reference to ~1e-6).
  Row sums S_i come for free from the `output` matmul by augmenting h_prime
  with a ones column (psum column 32 = sum_j e_ij).
"""
import os
import numpy as np
from contextlib import ExitStack

import concourse.bass as bass
import concourse.tile as tile
from concourse import bacc, mybir, bass_utils

N_HEAD, N, F_IN, F_OUT = 4, 6144, 64, 32
N_CORES = 8
ROWS = N // N_CORES          # 768 query rows per core
P = 128                      # partition dim
NBLK = ROWS // P             # 6 i-blocks per head per core
NCH = N // P                 # 48 key chunks of 128
FD = N                       # free dim of a full row tile
F32 = mybir.dt.float32

_CACHED_NC = None


def _build():
    nc = bacc.Bacc("TRN2", target_bir_lowering=False, debug=False,
                   num_devices=N_CORES)
    # ---- DRAM I/O ----
    h_d = nc.dram_tensor("h", (N, F_IN), F32, kind="ExternalInput").ap()
    hq_d = nc.dram_tensor("hq", (ROWS, F_IN), F32, kind="ExternalInput").ap()
    w_d = nc.dram_tensor("w", (N_HEAD, F_IN, F_OUT), F32, kind="ExternalInput").ap()
    asrc_d = nc.dram_tensor("a_src", (N_HEAD, F_OUT), F32, kind="ExternalInput").ap()
    adst_d = nc.dram_tensor("a_dst", (N_HEAD, F_OUT), F32, kind="ExternalInput").ap()
    bias_d = nc.dram_tensor("bias", (F_OUT,), F32, kind="ExternalInput").ap()
    ident_d = nc.dram_tensor("ident", (P, P), F32, kind="ExternalInput").ap()

    attn_d = nc.dram_tensor("attn_part", (N_HEAD, ROWS, N), F32,
                            kind="ExternalOutput").ap()
    out_d = nc.dram_tensor("out_part", (N_HEAD, ROWS, F_OUT), F32,
                           kind="ExternalOutput").ap()

    with tile.TileContext(nc) as tc, ExitStack() as ctx:
        # ---------------- persistent pools ----------------
        pers = ctx.enter_context(tc.tile_pool(name="pers", bufs=1))
        ident = pers.tile([P, P], F32)
        nc.sync.dma_start(ident[:], ident_d)

        # hp augmented with a ones column: per head [128, 48*33]
        hp_aug = [pers.tile([P, NCH * (F_OUT + 1)], F32, name=f"hp_aug{h}")
                  for h in range(N_HEAD)]
        # per head per i-block biases: src and 0.2*src, [128, NBLK]
        src_b = [pers.tile([P, NBLK], F32, name=f"src_b{h}") for h in range(N_HEAD)]
        src_b2 = [pers.tile([P, NBLK], F32, name=f"src_b2{h}") for h in range(N_HEAD)]
        # persistent hT (dst row is built in transient 512-chunks per head)
        hT = pers.tile([F_IN, N], F32)
        u_cols = [pers.tile([F_IN, 1], F32, name=f"u_col{h}") for h in range(N_HEAD)]
        # bias broadcast [128, 32]
        bias_bc = pers.tile([P, F_OUT], F32)

        # ---------------- main loop pools ----------------
        mp = ctx.enter_context(tc.tile_pool(name="mp", bufs=1))
        e_pool = ctx.enter_context(tc.tile_pool(name="e", bufs=3))
        pq_pool = ctx.enter_context(tc.tile_pool(name="pq", bufs=2))
        eT_pool = ctx.enter_context(tc.tile_pool(name="eT", bufs=2))
        small = ctx.enter_context(tc.tile_pool(name="small", bufs=4))

        # ---------------- prep ----------------
        with tc.tile_pool(name="prep", bufs=1) as prep, \
             tc.tile_pool(name="prep_ps", bufs=2, space="PSUM") as prep_ps:
            # hT [64, N]: transpose h chunks
            for c in range(NCH):
                htile = prep.tile([P, F_IN], F32, tag="h_load")
                nc.sync.dma_start(htile[:], h_d[c * P:(c + 1) * P, :])
                ps = prep_ps.tile([F_IN, P], F32, tag="hT_ps")
                nc.tensor.transpose(ps[:], htile[:], ident[:])
                if c % 2 == 0:
                    nc.scalar.copy(hT[:, c * P:(c + 1) * P], ps[:])
                else:
                    nc.vector.tensor_copy(hT[:, c * P:(c + 1) * P], ps[:])
            # hqT [64, ROWS]
            hqT = prep.tile([F_IN, ROWS], F32)
            for c in range(NBLK):
                htile = prep.tile([P, F_IN], F32, tag="h_load")
                nc.sync.dma_start(htile[:], hq_d[c * P:(c + 1) * P, :])
                ps = prep_ps.tile([F_IN, P], F32, tag="hT_ps")
                nc.tensor.transpose(ps[:], htile[:], ident[:])
                nc.scalar.copy(hqT[:, c * P:(c + 1) * P], ps[:])
            # ones column of hp_aug
            for h in range(N_HEAD):
                nc.vector.memset(
                    hp_aug[h][:].rearrange("p (c k) -> p c k", k=F_OUT + 1)[:, :, F_OUT:F_OUT + 1],
                    1.0)
            # bias broadcast via ones-outer: [1,32] -> [128,32]
            bias_row = prep.tile([1, F_OUT], F32)
            nc.sync.dma_start(bias_row[:], bias_d.unsqueeze(0))
            ones1 = prep.tile([1, P], F32)
            nc.vector.memset(ones1[:], 1.0)
            ps_b = prep_ps.tile([P, F_OUT], F32, tag="small_ps")
            nc.tensor.matmul(ps_b[:], ones1[:], bias_row[:], start=True, stop=True)
            nc.scalar.copy(bias_bc[:], ps_b[:])

            for h in range(N_HEAD):
                # w_h [64, 32]
                wt = prep.tile([F_IN, F_OUT], F32, tag="w_load")
                nc.sync.dma_start(wt[:], w_d[h])
                # hp chunks: lhsT = hT[:, chunk] [64,128], rhs = w_h -> [128, 32]
                for c in range(NCH):
                    ps = prep_ps.tile([P, F_OUT], F32, tag="hp_ps")
                    nc.tensor.matmul(ps[:], hT[:, c * P:(c + 1) * P], wt[:],
                                     start=True, stop=True)
                    if c % 2 == 0:
                        nc.scalar.copy(
                            hp_aug[h][:, c * (F_OUT + 1):c * (F_OUT + 1) + F_OUT],
                            ps[:])
                    else:
                        nc.vector.tensor_copy(
                            hp_aug[h][:, c * (F_OUT + 1):c * (F_OUT + 1) + F_OUT],
                            ps[:])
                # v = w_h @ a_src, u = w_h @ a_dst  (via wT [32, 64])
                ps_wT = prep_ps.tile([F_OUT, F_IN], F32, tag="small_ps")
                nc.tensor.transpose(ps_wT[:], wt[:], ident[0:F_IN, 0:F_IN])
                wT = prep.tile([F_OUT, F_IN], F32, tag="wT")
                nc.scalar.copy(wT[:], ps_wT[:])
                acol = prep.tile([F_OUT, 1], F32, tag="acol")
                nc.sync.dma_start(acol[:], asrc_d[h].unsqueeze(1))
                ps_v = prep_ps.tile([F_IN, 1], F32, tag="small_ps")
                nc.tensor.matmul(ps_v[:], wT[:], acol[:], start=True, stop=True)
                v = prep.tile([F_IN, 1], F32, tag="v")
                nc.scalar.copy(v[:], ps_v[:])
                acol2 = prep.tile([F_OUT, 1], F32, tag="acol2")
                nc.sync.dma_start(acol2[:], adst_d[h].unsqueeze(1))
                ps_u = prep_ps.tile([F_IN, 1], F32, tag="small_ps")
                nc.tensor.matmul(ps_u[:], wT[:], acol2[:], start=True, stop=True)
                nc.scalar.copy(u_cols[h][:], ps_u[:])
                # src per i-block: lhsT = hqT chunk [64, 128], rhs = v -> [128, 1]
                for b in range(NBLK):
                    ps = prep_ps.tile([P, 1], F32, tag="small_ps")
                    nc.tensor.matmul(ps[:], hqT[:, b * P:(b + 1) * P], v[:],
                                     start=True, stop=True)
                    nc.scalar.copy(src_b[h][:, b:b + 1], ps[:])
                nc.vector.tensor_scalar_mul(src_b2[h][:], src_b[h][:], 0.2)

        ps_tp = ctx.enter_context(tc.tile_pool(name="ps_tp", bufs=2, space="PSUM"))
        ps_out = ctx.enter_context(tc.tile_pool(name="ps_out", bufs=2, space="PSUM"))
        ps_bc = ctx.enter_context(tc.tile_pool(name="ps_bc", bufs=1, space="PSUM"))
        dst_bc = mp.tile([P, N], F32)          # per-head broadcast of dst row
        ones1m = mp.tile([1, P], F32)
        nc.vector.memset(ones1m[:], 1.0)

        JC = 1536  # j-chunk size for p/q temporaries

        for h in range(N_HEAD):
            # dst row chunk ([1,512]) then broadcast to [128,512] via ones-outer
            for c in range(N // 512):
                ps_d = ps_bc.tile([1, 512], F32, tag="dstps")
                nc.tensor.matmul(ps_d[:], u_cols[h][:], hT[:, c * 512:(c + 1) * 512],
                                 start=True, stop=True)
                drow = small.tile([1, 512], F32, tag="drow", bufs=2)
                nc.scalar.copy(drow[:], ps_d[:])
                ps = ps_bc.tile([P, 512], F32, tag="bc")
                nc.tensor.matmul(ps[:], ones1m[:], drow[:],
                                 start=True, stop=True)
                eng = nc.scalar if c % 2 == 0 else nc.vector
                if c % 2 == 0:
                    nc.scalar.copy(dst_bc[:, c * 512:(c + 1) * 512], ps[:])
                else:
                    nc.vector.tensor_copy(dst_bc[:, c * 512:(c + 1) * 512], ps[:])

            for b in range(NBLK):
                e_row = e_pool.tile([P, N], F32, tag="e_row")
                # generate e = max(exp(x), exp(0.2 x)), x = src_i + dst_j
                for jc in range(N // JC):
                    sl = slice(jc * JC, (jc + 1) * JC)
                    p_t = pq_pool.tile([P, JC], F32, tag="p_t")
                    nc.scalar.activation(p_t[:], dst_bc[:, sl],
                                         mybir.ActivationFunctionType.Exp,
                                         bias=src_b[h][:, b:b + 1], scale=1.0)
                    q_t = pq_pool.tile([P, JC], F32, tag="q_t")
                    nc.scalar.activation(q_t[:], dst_bc[:, sl],
                                         mybir.ActivationFunctionType.Exp,
                                         bias=src_b2[h][:, b:b + 1], scale=0.2)
                    nc.vector.tensor_tensor(e_row[:, sl], p_t[:], q_t[:],
                                            mybir.AluOpType.max)
                # transpose e chunks, evict, accumulate out-matmul
                ps_o = ps_out.tile([P, F_OUT + 1], F32, tag="po")
                for g in range(NCH // 8):     # groups of 8 transposes -> [128, 1024]
                    ps_t = ps_tp.tile([P, 1024], F32, tag="pt")
                    for k in range(8):
                        c = g * 8 + k
                        nc.tensor.transpose(ps_t[:, k * P:(k + 1) * P],
                                            e_row[:, c * P:(c + 1) * P], ident[:])
                    eT = eT_pool.tile([P, 1024], F32, tag="eT")
                    if g % 2 == 0:
                        nc.scalar.copy(eT[:], ps_t[:])
                    else:
                        nc.vector.tensor_copy(eT[:], ps_t[:])
                    for k in range(8):
                        c = g * 8 + k
                        nc.tensor.matmul(
                            ps_o[:], eT[:, k * P:(k + 1) * P],
                            hp_aug[h][:, c * (F_OUT + 1):(c + 1) * (F_OUT + 1)],
                            start=(c == 0), stop=(c == NCH - 1))
                # S, r, normalize, outputs
                r = small.tile([P, 1], F32, tag="r")
                nc.vector.reciprocal(r[:], ps_o[:, F_OUT:F_OUT + 1])
                nc.vector.tensor_scalar(e_row[:], e_row[:], r[:], None,
                                        mybir.AluOpType.mult)
                nc.sync.dma_start(attn_d[h, b * P:(b + 1) * P, :], e_row[:])
                out_sc = small.tile([P, F_OUT], F32, tag="out_sc")
                nc.vector.tensor_scalar(out_sc[:], ps_o[:, 0:F_OUT], r[:], None,
                                        mybir.AluOpType.mult)
                out_fin = small.tile([P, F_OUT], F32, tag="out_fin")
                nc.vector.tensor_tensor(out_fin[:], out_sc[:], bias_bc[:],
                                        mybir.AluOpType.add)
                nc.sync.dma_start(out_d[h, b * P:(b + 1) * P, :], out_fin[:])

    nc.compile()
    return nc


def kernel(h, w, a_src, a_dst, bias):
    global _CACHED_NC
    h = np.ascontiguousarray(h, dtype=np.float32)
    w = np.ascontiguousarray(w, dtype=np.float32)
    a_src = np.ascontiguousarray(a_src, dtype=np.float32)
    a_dst = np.ascontiguousarray(a_dst, dtype=np.float32)
    bias = np.ascontiguousarray(bias, dtype=np.float32)

    if _CACHED_NC is None:
        _CACHED_NC = _build()
    nc = _CACHED_NC

    ident = np.eye(P, dtype=np.float32)
    in_maps = []
    for c in range(N_CORES):
        in_maps.append({
            "h": h,
            "hq": h[c * ROWS:(c + 1) * ROWS],
            "w": w,
            "a_src": a_src,
            "a_dst": a_dst,
            "bias": bias,
            "ident": ident,
        })
    res = bass_utils.run_bass_kernel_spmd(nc, in_maps, core_ids=list(range(N_CORES)))
    attn = np.concatenate([r["attn_part"] for r in res.results], axis=1)
    output = np.concatenate([r["out_part"] for r in res.results], axis=1)
    return (output, attn)


if __name__ == "__main__":
    # quick self-run with random inputs
    rng = np.random.default_rng(0)
    h = rng.standard_normal((N, F_IN)).astype(np.float32)
    w = (rng.standard_normal((N_HEAD, F_IN, F_OUT)) * 0.15).astype(np.float32)
    a_src = (rng.standard_normal((N_HEAD, F_OUT)) * 0.2).astype(np.float32)
    a_dst = (rng.standard_normal((N_HEAD, F_OUT)) * 0.2).astype(np.float32)
    bias = np.zeros(F_OUT, dtype=np.float32)
    out, attn = kernel(h=h, w=w, a_src=a_src, a_dst=a_dst, bias=bias)
    print("out", out.shape, "attn", attn.shape, attn[0, 0, :4])
